# revision 6
# baseline (speedup 1.0000x reference)
"""DRQN forward kernel for Trainium2 (Bass/Tile), 8-core data-parallel.

Model (per timestep t, batch B):
  a  = relu(relu(x_t @ W1a.T + b1a) @ W1b.T + b1b)
  gi = a @ W_ih.T + b_ih                  # [B, 3H] precomputable for all t
  gh = h @ W_hh.T + b_hh                  # sequential in t
  r, z = sigmoid(gi_rz + gh_rz); n = tanh(gi_n + r * gh_n)
  h' = (1 - z) * n + z * h
  q  = relu(h' @ W2a.T + b2a) @ W2b.T + b2b

Sharding: data-parallel over batch across 8 cores (32 sequences each),
weights replicated, no collectives.  Layout on chip is feature-major
("transposed"): every activation lives as [feature-partition, (t*batch)
free] so GEMM epilogues can use per-partition biases and the GRU gate
math runs on fully-populated 128-lane tiles.
"""

import numpy as np
from contextlib import ExitStack

import concourse.bass as bass
import concourse.bacc as bacc
import concourse.mybir as mybir
import concourse.tile as tile

F32 = mybir.dt.float32
BF16 = mybir.dt.bfloat16
AF = mybir.ActivationFunctionType
ALU = mybir.AluOpType

IN_DIM = 256
H1 = 512
H = 256
G = 3 * H  # 768
OUT_DIM = 18

N_CORES = 8
T_FULL = 512
B_FULL = 256
B_LOC = B_FULL // N_CORES  # 32


def build_drqn(T=T_FULL, B=B_LOC, TC=32):
    """Build the single-core Bass program (SPMD-replicated across cores)."""
    assert T % TC == 0
    NCH = T // TC
    N = TC * B  # free-dim length of one chunk, (t, b) flattened

    nc = bacc.Bacc("TRN2", debug=False)

    x_d = nc.dram_tensor("x", [T, B, IN_DIM], F32, kind="ExternalInput")
    w1a_d = nc.dram_tensor("W_fc1a", [H1, IN_DIM], F32, kind="ExternalInput")
    b1a_d = nc.dram_tensor("b_fc1a", [H1], F32, kind="ExternalInput")
    w1b_d = nc.dram_tensor("W_fc1b", [H, H1], F32, kind="ExternalInput")
    b1b_d = nc.dram_tensor("b_fc1b", [H], F32, kind="ExternalInput")
    wih_d = nc.dram_tensor("W_ih", [G, H], F32, kind="ExternalInput")
    bih_d = nc.dram_tensor("b_ih", [G], F32, kind="ExternalInput")
    whh_d = nc.dram_tensor("W_hh", [G, H], F32, kind="ExternalInput")
    bhh_d = nc.dram_tensor("b_hh", [G], F32, kind="ExternalInput")
    w2a_d = nc.dram_tensor("W_fc2a", [H1, H], F32, kind="ExternalInput")
    b2a_d = nc.dram_tensor("b_fc2a", [H1], F32, kind="ExternalInput")
    w2b_d = nc.dram_tensor("W_fc2b", [OUT_DIM, H1], F32, kind="ExternalInput")
    b2b_d = nc.dram_tensor("b_fc2b", [OUT_DIM], F32, kind="ExternalInput")
    q_d = nc.dram_tensor("q", [T, B, OUT_DIM], F32, kind="ExternalOutput")

    with tile.TileContext(nc) as tc, ExitStack() as ctx:
        wpool = ctx.enter_context(tc.tile_pool(name="weights", bufs=1))
        stage = ctx.enter_context(tc.tile_pool(name="stage", bufs=2))
        xpool = ctx.enter_context(tc.tile_pool(name="x", bufs=2))
        apool = ctx.enter_context(tc.tile_pool(name="acts", bufs=2))
        gipool = ctx.enter_context(tc.tile_pool(name="gi", bufs=2))
        hpool = ctx.enter_context(tc.tile_pool(name="h", bufs=2))
        qpool = ctx.enter_context(tc.tile_pool(name="q", bufs=2))
        gpool = ctx.enter_context(tc.tile_pool(name="gates", bufs=3))
        ps = ctx.enter_context(tc.tile_pool(name="ps", bufs=2, space="PSUM"))
        ps_rz = ctx.enter_context(tc.tile_pool(name="ps_rz", bufs=2, space="PSUM"))
        ps_n = ctx.enter_context(tc.tile_pool(name="ps_n", bufs=2, space="PSUM"))

        # ---- weight / bias staging (fp32 DMA in transposed layout -> bf16) --
        def load_wT(w_dram, K, M):
            """Return list over k-tiles of [128, M] bf16 lhsT tiles (W^T)."""
            tiles = []
            for k in range(K // 128):
                t_f32 = stage.tile([128, M], F32, tag="wstage")
                src = w_dram[:, k * 128:(k + 1) * 128].rearrange("m k -> k m")
                nc.sync.dma_start(t_f32[:], src)
                t_bf = wpool.tile([128, M], BF16, tag=f"w_{w_dram.name}_{k}")
                nc.vector.tensor_copy(t_bf[:], t_f32[:])
                tiles.append(t_bf)
            return tiles

        w1aT = load_wT(w1a_d, IN_DIM, H1)   # 2 x [128, 512]
        w1bT = load_wT(w1b_d, H1, H)        # 4 x [128, 256]
        wihT = load_wT(wih_d, H, G)         # 2 x [128, 768]
        whhT = load_wT(whh_d, H, G)         # 2 x [128, 768]
        w2aT = load_wT(w2a_d, H, H1)        # 2 x [128, 512]
        w2bT = load_wT(w2b_d, H1, OUT_DIM)  # 4 x [128, 18]

        def load_bias(b_dram, M):
            tiles = []
            for m in range(max(1, M // 128)):
                lo = m * 128
                hi = min(M, lo + 128)
                t = wpool.tile([hi - lo, 1], F32, tag=f"bias_{b_dram.name}_{m}")
                nc.sync.dma_start(t[:], b_dram[lo:hi].rearrange("(p o) -> p o", o=1))
                tiles.append(t)
            return tiles

        b1a = load_bias(b1a_d, H1)
        b1b = load_bias(b1b_d, H)
        bih = load_bias(bih_d, G)           # 6 x [128, 1]
        bhh = load_bias(bhh_d, G)           # 6 x [128, 1]
        b2a = load_bias(b2a_d, H1)
        b2b = load_bias(b2b_d, OUT_DIM)     # 1 x [18, 1]

        # gi epilogue bias: b_ih + b_hh for r,z blocks (j<4); b_ih only for n
        gi_bias = []
        for j in range(6):
            if j < 4:
                t = wpool.tile([128, 1], F32, tag=f"gib{j}")
                nc.vector.tensor_add(t[:], bih[j][:], bhh[j][:])
                gi_bias.append(t)
            else:
                gi_bias.append(bih[j])

        # broadcast b_hh for the n blocks to a [128, 2*B] tile (cols j4|j5)
        bhh_bc = wpool.tile([128, 2 * B], F32)
        zero_bf = wpool.tile([128, 2 * B], BF16)
        nc.vector.memset(zero_bf[:], 0.0)
        zero_f32 = wpool.tile([128, 2 * B], F32)
        nc.vector.memset(zero_f32[:], 0.0)
        nc.vector.tensor_scalar_add(bhh_bc[:, 0:B], zero_f32[:, 0:B], bhh[4][:])
        nc.vector.tensor_scalar_add(bhh_bc[:, B:2 * B], zero_f32[:, B:2 * B], bhh[5][:])

        # ---------------- GEMM helper (feature-major) -----------------------
        NT = 512  # psum free tile

        def gemm(lhsT, rhs_fn, M, out_fn, n_len):
            """out[m-tile][:, n0:n1] = epilogue(sum_k lhsT[k][:,m-slice].T @ rhs(k))"""
            n_m = (M + 127) // 128
            for mi in range(n_m):
                mlo = mi * 128
                mhi = min(M, mlo + 128)
                for ni in range(0, n_len, NT):
                    nhi = min(n_len, ni + NT)
                    p = ps.tile([mhi - mlo, NT], F32, tag="gemm")
                    for k in range(len(lhsT)):
                        nc.tensor.matmul(
                            p[:, 0:nhi - ni],
                            lhsT[k][:, mlo:mhi],
                            rhs_fn(k)[:, ni:nhi],
                            start=(k == 0),
                            stop=(k == len(lhsT) - 1),
                        )
                    out_fn(p[:, 0:nhi - ni], mi, ni, nhi)

        # ---------------- per-chunk pipeline --------------------------------
        h_prev_bf = None   # AP source for h rhs slices of previous step
        h_prev_f32 = None  # [128, 2B] f32 tile of previous step
        hs_prev = None     # previous chunk's h_sb (keep alive)

        for c in range(NCH):
            t0, t1 = c * TC, (c + 1) * TC

            # -- load + cast x chunk: [128, 2, N] feature-major bf16
            x_src = x_d[t0:t1].rearrange("t b k -> k (t b)")
            xbf = xpool.tile([128, 2, N], BF16, tag="xbf")
            for k in range(2):
                xf = stage.tile([128, N], F32, tag="xstage")
                nc.sync.dma_start(xf[:], x_src[k * 128:(k + 1) * 128, :])
                nc.vector.tensor_copy(xbf[:, k, :], xf[:])

            # -- fc1a: a1 = relu(W1a @ x + b1a)   [128, 4, N] bf16
            a1 = apool.tile([128, 4, N], BF16, tag="a1")

            def ep_relu(dst, btiles):
                def ep(p, mi, ni, nhi):
                    if (ni // NT) % 2 == 0:
                        nc.scalar.activation(dst[:, mi, ni:nhi], p, AF.Relu,
                                             bias=btiles[mi][:])
                    else:
                        nc.vector.tensor_scalar(dst[:, mi, ni:nhi], p,
                                                btiles[mi][:], 0.0,
                                                ALU.add, ALU.max)
                return ep

            gemm(w1aT, lambda k: xbf[:, k, :], H1, ep_relu(a1, b1a), N)

            # -- fc1b: a = relu(W1b @ a1 + b1b)   [128, 2, N] bf16
            a_sb = apool.tile([128, 2, N], BF16, tag="a")
            gemm(w1bT, lambda k: a1[:, k, :], H, ep_relu(a_sb, b1b), N)

            # -- gi = W_ih @ a + (b_ih [+ b_hh])  [128, 6, N] bf16
            gi = gipool.tile([128, 6, N], BF16, tag="gi")

            def ep_gi(p, mi, ni, nhi):
                nc.vector.tensor_scalar_add(gi[:, mi, ni:nhi], p, gi_bias[mi][:])

            gemm(wihT, lambda k: a_sb[:, k, :], G, ep_gi, N)

            # -- GRU scan over this chunk
            h_sb = hpool.tile([128, 2, N], BF16, tag="hchunk")
            for ti in range(TC):
                t = t0 + ti
                cb = ti * B  # column base
                gi_rz = gi[:, 0:4, cb:cb + B]   # [128, (4,B)]
                gi_n = gi[:, 4:6, cb:cb + B]    # [128, (2,B)]

                if t == 0:
                    # h == 0: gh == 0
                    rz = gpool.tile([128, 4 * B], F32, tag="rz")
                    nc.scalar.activation(rz[:, 0:4 * B], gi_rz, AF.Sigmoid)
                    w_n = bhh_bc
                else:
                    prz = ps_rz.tile([128, 4 * B], F32, tag="ps_rz")
                    pn = ps_n.tile([128, 2 * B], F32, tag="ps_n")
                    # r,z feature blocks first so sigmoid can start early
                    for j in range(6):
                        dst = (prz[:, j * B:(j + 1) * B] if j < 4
                               else pn[:, (j - 4) * B:(j - 3) * B])
                        for k in range(2):
                            nc.tensor.matmul(
                                dst,
                                whhT[k][:, j * 128:(j + 1) * 128],
                                h_prev_bf(k),
                                start=(k == 0),
                                stop=(k == 1),
                            )
                    s_rz = gpool.tile([128, 4 * B], F32, tag="s_rz")
                    nc.vector.tensor_add(s_rz[:], prz[:], gi_rz)
                    rz = gpool.tile([128, 4 * B], F32, tag="rz")
                    nc.scalar.activation(rz[:], s_rz[:], AF.Sigmoid)
                    w_n = gpool.tile([128, 2 * B], F32, tag="w_n")
                    nc.vector.tensor_add(w_n[:], pn[:], bhh_bc[:])

                r_ap = rz[:, 0:2 * B]
                z_ap = rz[:, 2 * B:4 * B]
                u = gpool.tile([128, 2 * B], F32, tag="u")
                nc.vector.tensor_mul(u[:], r_ap, w_n[:])
                v = gpool.tile([128, 2 * B], F32, tag="v")
                nc.vector.tensor_add(v[:], u[:], gi_n)
                n_t = gpool.tile([128, 2 * B], F32, tag="n_t")
                nc.scalar.activation(n_t[:], v[:], AF.Tanh)

                # zh = z * h_prev (0 at t=0), off critical path
                zh = gpool.tile([128, 2 * B], F32, tag="zh")
                if t == 0:
                    nc.vector.memset(zh[:], 0.0)
                else:
                    nc.vector.tensor_mul(zh[:], z_ap, h_prev_f32[:])
                zn = gpool.tile([128, 2 * B], F32, tag="zn")
                nc.vector.tensor_mul(zn[:], z_ap, n_t[:])
                nmzn = gpool.tile([128, 2 * B], F32, tag="nmzn")
                nc.vector.tensor_sub(nmzn[:], n_t[:], zn[:])
                # h' = n - z*n + z*h  -> bf16 into the chunk tensor (chain)
                nc.vector.tensor_add(h_sb[:, :, cb:cb + B], nmzn[:], zh[:])
                hf = gpool.tile([128, 2 * B], F32, tag="hf")
                nc.vector.tensor_add(hf[:], nmzn[:], zh[:])

                hs_cur = h_sb

                def mk_rhs(hs=hs_cur, col=cb):
                    return lambda k: hs[:, k, col:col + B]

                h_prev_bf = mk_rhs()
                h_prev_f32 = hf

            # -- fc2a: a2 = relu(W2a @ h + b2a)  [128, 4, N] bf16
            a2 = apool.tile([128, 4, N], BF16, tag="a2")
            gemm(w2aT, lambda k: h_sb[:, k, :], H1, ep_relu(a2, b2a), N)

            # -- fc2b: q = W2b @ a2 + b2b        [18, N] f32
            q_sb = qpool.tile([OUT_DIM, N], F32, tag="q")

            def ep_q(p, mi, ni, nhi):
                nc.vector.tensor_scalar_add(q_sb[:, ni:nhi], p, b2b[0][:])

            gemm(w2bT, lambda k: a2[:, k, :], OUT_DIM, ep_q, N)

            # -- store q chunk
            q_dst = q_d[t0:t1].rearrange("t b o -> o (t b)")
            nc.sync.dma_start(q_dst, q_sb[:])

            hs_prev = h_sb  # noqa: F841  (keep previous chunk alive)

    nc.compile()
    return nc


def kernel(**inputs) -> np.ndarray:
    from concourse.bass_utils import run_bass_kernel_spmd

    x = np.asarray(inputs["x"], dtype=np.float32)
    T, B, _ = x.shape
    b_loc = B // N_CORES

    nc = build_drqn(T=T, B=b_loc, TC=min(32, T))

    weight_names = ["W_fc1a", "b_fc1a", "W_fc1b", "b_fc1b", "W_ih", "b_ih",
                    "W_hh", "b_hh", "W_fc2a", "b_fc2a", "W_fc2b", "b_fc2b"]
    weights = {k: np.ascontiguousarray(np.asarray(inputs[k], dtype=np.float32))
               for k in weight_names}

    in_maps = []
    for c in range(N_CORES):
        shard = np.ascontiguousarray(x[:, c * b_loc:(c + 1) * b_loc, :])
        in_maps.append({"x": shard, **weights})

    res = run_bass_kernel_spmd(nc, in_maps, list(range(N_CORES)))
    out = np.concatenate([r["q"] for r in res.results], axis=1)
    return out.astype(np.float32)


if __name__ == "__main__":
    rng = np.random.default_rng(0)
    s = 0.05
    T, B = 8, 256
    ins = {
        "x": rng.standard_normal((T, B, IN_DIM), dtype=np.float32),
        "W_fc1a": rng.standard_normal((H1, IN_DIM), dtype=np.float32) * s,
        "b_fc1a": rng.standard_normal((H1,), dtype=np.float32) * s,
        "W_fc1b": rng.standard_normal((H, H1), dtype=np.float32) * s,
        "b_fc1b": rng.standard_normal((H,), dtype=np.float32) * s,
        "W_ih": rng.standard_normal((G, H), dtype=np.float32) * s,
        "b_ih": rng.standard_normal((G,), dtype=np.float32) * s,
        "W_hh": rng.standard_normal((G, H), dtype=np.float32) * s,
        "b_hh": rng.standard_normal((G,), dtype=np.float32) * s,
        "W_fc2a": rng.standard_normal((H1, H), dtype=np.float32) * s,
        "b_fc2a": rng.standard_normal((H1,), dtype=np.float32) * s,
        "W_fc2b": rng.standard_normal((OUT_DIM, H1), dtype=np.float32) * s,
        "b_fc2b": rng.standard_normal((OUT_DIM,), dtype=np.float32) * s,
    }
    print("building tiny kernel...")
    # numpy reference
    def np_ref(ins):
        x = ins["x"]
        relu = lambda v: np.maximum(v, 0.0)
        sig = lambda v: 1.0 / (1.0 + np.exp(-v))
        h = np.zeros((x.shape[1], H), np.float32)
        qs = []
        for t in range(x.shape[0]):
            a = relu(relu(x[t] @ ins["W_fc1a"].T + ins["b_fc1a"]) @ ins["W_fc1b"].T + ins["b_fc1b"])
            gi = a @ ins["W_ih"].T + ins["b_ih"]
            gh = h @ ins["W_hh"].T + ins["b_hh"]
            i_r, i_z, i_n = np.split(gi, 3, -1)
            h_r, h_z, h_n = np.split(gh, 3, -1)
            r = sig(i_r + h_r)
            z = sig(i_z + h_z)
            n = np.tanh(i_n + r * h_n)
            h = (1 - z) * n + z * h
            qs.append(relu(h @ ins["W_fc2a"].T + ins["b_fc2a"]) @ ins["W_fc2b"].T + ins["b_fc2b"])
        return np.stack(qs)

    expected = np_ref(ins)
    actual = kernel(**ins)
    err = np.abs(actual - expected)
    rel = np.linalg.norm(actual - expected) / np.linalg.norm(expected)
    print("max abs err:", err.max(), "rel l2:", rel)


# revision 19
# speedup vs baseline: 63.2541x; 63.2541x over previous
"""DRQN forward kernel for Trainium2 (Bass/Tile), 8-core data-parallel.

Model (per timestep t, batch B):
  a  = relu(relu(x_t @ W1a.T + b1a) @ W1b.T + b1b)
  gi = a @ W_ih.T + b_ih                  # [B, 3H] precomputable for all t
  gh = h @ W_hh.T + b_hh                  # sequential in t
  r, z = sigmoid(gi_rz + gh_rz); n = tanh(gi_n + r * gh_n)
  h' = (1 - z) * n + z * h
  q  = relu(h' @ W2a.T + b2a) @ W2b.T + b2b

Sharding: data-parallel over batch across 8 cores (32 sequences each),
weights replicated, no collectives.  On-chip layout is feature-major:
activations live as [feature-partition, (t*batch) free] so GEMM epilogues
use per-partition biases and GRU gate math runs on full 128-lane tiles.

The T-sequential GRU scan is latency-bound (~2-3us/step); the batched
GEMMs of the next/previous chunk are hand-interleaved into the scan's
per-engine instruction streams so they execute inside the scan's stalls.
Tricks: gi is injected into PSUM by an identity matmul (removes a DVE add
from the chain), b_hh of the n-gate enters via a tiny indicator matmul,
and (1-z)*n is one fused custom DVE op.
"""

import numpy as np
from contextlib import ExitStack

import concourse.bass as bass
import concourse.bacc as bacc
import concourse.mybir as mybir
import concourse.tile as tile

F32 = mybir.dt.float32
BF16 = mybir.dt.bfloat16
I32 = mybir.dt.int32
AF = mybir.ActivationFunctionType
ALU = mybir.AluOpType

IN_DIM = 256
H1 = 512
H = 256
G = 3 * H  # 768
OUT_DIM = 18

N_CORES = 8
T_FULL = 512
B_FULL = 256
B_LOC = B_FULL // N_CORES  # 32


# --------------------------------------------------------------------------
# custom DVE op: out = (1 - Src1) * Src0      (n, z) -> (1-z)*n
# --------------------------------------------------------------------------
def _ensure_gru_blend():
    from concourse import dve_ops as dops
    from concourse.dve_spec import Spec, Src0, Src1, One, lower, _has_src1 as has_src1
    from concourse.dve_uop import DveOpSpec

    for o in dops.OPS:
        if o.name == "GRU_BLEND_ANT":
            return o
    spec = Spec(
        body=(One - Src1) * Src0,
        reference=lambda in0, in1, c0, c1, c2: (1.0 - in1) * in0,
    )
    opcode = dops._CUSTOM_DVE_ROW_BASE + len(dops.OPS)
    shas = {}
    for ver in ("v3", "v4"):
        try:
            tmp = DveOpSpec(name="GRU_BLEND_ANT", opcode=opcode,
                            uops=lower(spec, ver=ver), rd1_en=has_src1(spec))
            shas[ver] = tmp.sha(ver)
        except Exception:
            pass
    op = dops.DveOp("GRU_BLEND_ANT", spec, subdim=False, uops_sha=shas)
    dops.OPS.append(op)
    dops.CUSTOM_DVE_SPECS[op.name] = op.spec
    dops._SUB_OPCODE_FOR_NAME[op.name] = opcode
    return op


def build_drqn(T=T_FULL, B=B_LOC, TC=32):
    """Build the single-core Bass program (SPMD-replicated across cores)."""
    assert T % TC == 0
    NCH = T // TC
    N = TC * B  # free-dim length of one chunk, (t, b) flattened
    NT = 512    # psum free tile for GEMMs
    EPW = 256   # epilogue sub-op width (bounds chain-insertion damage)

    blend_op = _ensure_gru_blend()

    nc = bacc.Bacc("TRN2", debug=False)

    # all inputs are host-side pre-transposed so every DMA is contiguous
    x_d = nc.dram_tensor("xT", [IN_DIM, T * B], F32, kind="ExternalInput")
    w1a_d = nc.dram_tensor("W1aT", [IN_DIM, H1], F32, kind="ExternalInput")
    b1a_d = nc.dram_tensor("b_fc1a", [H1], F32, kind="ExternalInput")
    w1b_d = nc.dram_tensor("W1bT", [H1, H], F32, kind="ExternalInput")
    b1b_d = nc.dram_tensor("b_fc1b", [H], F32, kind="ExternalInput")
    wih_d = nc.dram_tensor("WihT", [H, G], F32, kind="ExternalInput")
    bih_d = nc.dram_tensor("b_ih", [G], F32, kind="ExternalInput")
    whh_d = nc.dram_tensor("WhhT", [H, G], F32, kind="ExternalInput")
    bhh_d = nc.dram_tensor("b_hh", [G], F32, kind="ExternalInput")
    w2a_d = nc.dram_tensor("W2aT", [H, H1], F32, kind="ExternalInput")
    b2a_d = nc.dram_tensor("b_fc2a", [H1], F32, kind="ExternalInput")
    w2b_d = nc.dram_tensor("W2bT", [H1, OUT_DIM], F32, kind="ExternalInput")
    b2b_d = nc.dram_tensor("b_fc2b", [OUT_DIM], F32, kind="ExternalInput")
    q_d = nc.dram_tensor("qT", [OUT_DIM, T * B], F32, kind="ExternalOutput")

    with tile.TileContext(nc) as tc, ExitStack() as ctx:
        wpool = ctx.enter_context(tc.tile_pool(name="weights", bufs=1))
        stage = ctx.enter_context(tc.tile_pool(name="stage", bufs=2))
        xpool = ctx.enter_context(tc.tile_pool(name="x", bufs=2))
        xstage = ctx.enter_context(tc.tile_pool(name="xstage", bufs=3))
        apool = ctx.enter_context(tc.tile_pool(name="acts", bufs=2))
        gipool = ctx.enter_context(tc.tile_pool(name="gi", bufs=2))
        hpool = ctx.enter_context(tc.tile_pool(name="h", bufs=2))
        qpool = ctx.enter_context(tc.tile_pool(name="q", bufs=2))
        gpool = ctx.enter_context(tc.tile_pool(name="gates", bufs=3))
        ps = ctx.enter_context(tc.tile_pool(name="ps", bufs=3, space="PSUM"))
        psrz = ctx.enter_context(tc.tile_pool(name="psrz", bufs=2, space="PSUM"))
        psn = ctx.enter_context(tc.tile_pool(name="psn", bufs=2, space="PSUM"))

        # ---- weight / bias staging (fp32 DMA in transposed layout -> bf16) -
        def load_wT(w_dram, K, M):
            tiles = []
            for k in range(K // 128):
                t_f32 = stage.tile([128, M], F32, tag="wstage")
                nc.sync.dma_start(t_f32[:], w_dram[k * 128:(k + 1) * 128, :])
                t_bf = wpool.tile([128, M], BF16, tag=f"w_{w_dram.name}_{k}")
                nc.vector.tensor_copy(t_bf[:], t_f32[:])
                tiles.append(t_bf)
            return tiles

        w1aT = load_wT(w1a_d, IN_DIM, H1)   # 2 x [128, 512]
        w1bT = load_wT(w1b_d, H1, H)        # 4 x [128, 256]
        wihT = load_wT(wih_d, H, G)         # 2 x [128, 768]
        whhT = load_wT(whh_d, H, G)         # 2 x [128, 768]
        w2aT = load_wT(w2a_d, H, H1)        # 2 x [128, 512]
        w2bT = load_wT(w2b_d, H1, OUT_DIM)  # 4 x [128, 18]

        def load_bias(b_dram, M):
            tiles = []
            for m in range(max(1, M // 128)):
                lo = m * 128
                hi = min(M, lo + 128)
                t = wpool.tile([hi - lo, 1], F32, tag=f"b_{b_dram.name}_{m}")
                nc.sync.dma_start(t[:], b_dram[lo:hi].rearrange("(p o) -> p o", o=1))
                tiles.append(t)
            return tiles

        b1a = load_bias(b1a_d, H1)
        b1b = load_bias(b1b_d, H)
        bih = load_bias(bih_d, G)           # 6 x [128, 1]
        bhh = load_bias(bhh_d, G)           # 6 x [128, 1]
        b2a = load_bias(b2a_d, H1)
        b2b = load_bias(b2b_d, OUT_DIM)     # 1 x [18, 1]

        # gi epilogue bias: b_ih + b_hh for r,z blocks (j<4); b_ih only for n
        gi_bias = []
        for j in range(6):
            if j < 4:
                t = wpool.tile([128, 1], F32, tag=f"gib{j}")
                nc.vector.tensor_add(t[:], bih[j][:], bhh[j][:])
                gi_bias.append(t)
            else:
                gi_bias.append(bih[j])

        # b_hh broadcast for the n blocks, [128, 2B] f32 (used at t==0)
        zero_f32 = wpool.tile([128, 2 * B], F32)
        nc.vector.memset(zero_f32[:], 0.0)
        bhh_bc = wpool.tile([128, 2 * B], F32)
        nc.vector.tensor_scalar_add(bhh_bc[:, 0:B], zero_f32[:, 0:B], bhh[4][:])
        nc.vector.tensor_scalar_add(bhh_bc[:, B:2 * B], zero_f32[:, B:2 * B], bhh[5][:])

        # identity [128,128] bf16 for the gi->PSUM inject matmul
        iota_j = wpool.tile([128, 128], I32)
        nc.gpsimd.iota(iota_j[:], pattern=[[1, 128]], base=0, channel_multiplier=0)
        iota_p = wpool.tile([128, 1], I32)
        nc.gpsimd.iota(iota_p[:], pattern=[[0, 1]], base=0, channel_multiplier=1)
        iota_jf = wpool.tile([128, 128], F32)
        nc.vector.tensor_copy(iota_jf[:], iota_j[:])
        iota_pf = wpool.tile([128, 1], F32)
        nc.vector.tensor_copy(iota_pf[:], iota_p[:])
        ident = wpool.tile([128, 128], BF16)
        nc.vector.tensor_scalar(ident[:], iota_jf[:], iota_pf[:], None, ALU.is_equal)

        # b_hh n-gate rows as a [2, 128] lhsT + [2, 2B] indicator rhs
        bhh2_f = stage.tile([2, 128], F32, tag="bhh2s")
        nc.sync.dma_start(bhh2_f[:], bhh_d[2 * H:3 * H].rearrange("(k m) -> k m", k=2))
        bhh2 = wpool.tile([2, 128], BF16)
        nc.vector.tensor_copy(bhh2[:], bhh2_f[:])
        # indicator ind2[k, col] = 1.0 iff col // B == k, built via iota
        ind_i = wpool.tile([2, 2 * B], I32)
        nc.gpsimd.iota(ind_i[:], pattern=[[1, 2 * B]], base=0,
                       channel_multiplier=-B)
        ind_f = wpool.tile([2, 2 * B], F32)
        nc.vector.tensor_copy(ind_f[:], ind_i[:])
        ind_lo = wpool.tile([2, 2 * B], F32)
        nc.vector.tensor_scalar(ind_lo[:], ind_f[:], -0.5, None, ALU.is_ge)
        ind_hi = wpool.tile([2, 2 * B], F32)
        nc.vector.tensor_scalar(ind_hi[:], ind_f[:], B - 0.5, None, ALU.is_le)
        ind2 = wpool.tile([2, 2 * B], BF16)
        nc.vector.tensor_mul(ind2[:], ind_lo[:], ind_hi[:])

        # ------------------------------------------------------------------
        # phase-1 / phase-3 GEMMs as interleavable quanta
        # ------------------------------------------------------------------
        def gemm_quanta(lhsT, rhs_fn, M, ep):
            """Yield closures; each emits one [<=128, NT] psum group + epilogue."""
            out = []
            for mi in range((M + 127) // 128):
                mlo, mhi = mi * 128, min(M, mi * 128 + 128)
                for ni in range(0, N, NT):
                    nhi = min(N, ni + NT)

                    def f(mi=mi, mlo=mlo, mhi=mhi, ni=ni, nhi=nhi):
                        p = ps.tile([mhi - mlo, NT], F32, tag="gemm")
                        for k in range(len(lhsT)):
                            nc.tensor.matmul(
                                p[:, 0:nhi - ni],
                                lhsT[k][:, mlo:mhi],
                                rhs_fn(k)[:, ni:nhi],
                                start=(k == 0),
                                stop=(k == len(lhsT) - 1),
                            )
                        ep(p[:, 0:nhi - ni], mi, ni, nhi)
                    out.append(f)
            return out

        def ep_relu_act(dst, btiles):
            def ep(p, mi, ni, nhi):
                nc.scalar.activation(dst[:, mi, ni:nhi], p, AF.Relu,
                                     bias=btiles[mi][:])
            return ep

        def ep_bias_dve(dst_fn, btiles):
            def ep(p, mi, ni, nhi):
                nc.vector.tensor_scalar_add(dst_fn(mi, ni, nhi), p, btiles[mi][:])
            return ep

        p1_state = {}  # chunk -> dict with xbf/a1/a/gi tiles
        h_tiles = {}   # chunk -> h_sb tile

        def make_xdma(c):
            """DMA the x chunk into fp32 staging (issued 2 chunks ahead)."""
            st = p1_state.setdefault(c, {})
            t0 = c * TC
            x_src = x_d[:, t0 * B:(t0 + TC) * B]

            def q_dma(k):
                def f():
                    if "xf" not in st:
                        st["xf"] = xstage.tile([128, 2, N], F32, tag="xf", name="xf")
                    nc.sync.dma_start(st["xf"][:, k, :],
                                      x_src[k * 128:(k + 1) * 128, :])
                return f
            return [q_dma(0), q_dma(1)]

        def make_phase1(c):
            st = p1_state.setdefault(c, {})
            quanta = []

            def q_cast(k, half):
                def f():
                    if "xbf" not in st:
                        st["xbf"] = xpool.tile([128, 2, N], BF16, tag="xbf", name="xbf")
                    lo, hi = half * (N // 2), (half + 1) * (N // 2)
                    nc.vector.tensor_copy(st["xbf"][:, k, lo:hi],
                                          st["xf"][:, k, lo:hi])
                return f

            quanta += [q_cast(0, 0), q_cast(0, 1), q_cast(1, 0), q_cast(1, 1)]

            def alloc(name, shape, pool, tag):
                def g():
                    if name not in st:
                        st[name] = pool.tile(shape, BF16, tag=tag, name=name)
                    return st[name]
                return g

            a1_t = alloc("a1", [128, 4, N], apool, "a1")
            a_t = alloc("a", [128, 2, N], apool, "a")
            gi_t = alloc("gi", [128, 6, N], gipool, "gi")

            def ep_a1(p, mi, ni, nhi):
                for s in range(0, nhi - ni, EPW):
                    e = min(nhi - ni, s + EPW)
                    if (mi + ni // NT + s // EPW) % 2 == 0:
                        nc.scalar.activation(a1_t()[:, mi, ni + s:ni + e],
                                             p[:, s:e], AF.Relu, bias=b1a[mi][:])
                    else:
                        nc.vector.tensor_scalar(a1_t()[:, mi, ni + s:ni + e],
                                                p[:, s:e], b1a[mi][:], 0.0,
                                                ALU.add, ALU.max)

            def ep_a(p, mi, ni, nhi):
                for s in range(0, nhi - ni, EPW):
                    e = min(nhi - ni, s + EPW)
                    if (mi + ni // NT + s // EPW) % 2 == 0:
                        nc.scalar.activation(a_t()[:, mi, ni + s:ni + e],
                                             p[:, s:e], AF.Relu, bias=b1b[mi][:])
                    else:
                        nc.vector.tensor_scalar(a_t()[:, mi, ni + s:ni + e],
                                                p[:, s:e], b1b[mi][:], 0.0,
                                                ALU.add, ALU.max)

            def ep_gi(p, mi, ni, nhi):
                for s in range(0, nhi - ni, EPW):
                    e = min(nhi - ni, s + EPW)
                    if (mi + ni // NT + s // EPW) % 2 == 0:
                        nc.vector.tensor_scalar_add(gi_t()[:, mi, ni + s:ni + e],
                                                    p[:, s:e], gi_bias[mi][:])
                    else:
                        nc.scalar.activation(gi_t()[:, mi, ni + s:ni + e],
                                             p[:, s:e], AF.Identity,
                                             bias=gi_bias[mi][:])

            quanta += gemm_quanta(w1aT, lambda k: st["xbf"][:, k, :], H1, ep_a1)
            quanta += gemm_quanta(w1bT, lambda k: st["a1"][:, k, :], H, ep_a)
            quanta += gemm_quanta(wihT, lambda k: st["a"][:, k, :], G, ep_gi)
            return quanta

        def make_phase3(c):
            st = {}
            t0 = c * TC
            quanta = []

            def a2_t():
                if "a2" not in st:
                    st["a2"] = apool.tile([128, 4, N], BF16, tag="a2", name="a2")
                return st["a2"]

            def q_t():
                if "q" not in st:
                    st["q"] = qpool.tile([OUT_DIM, N], F32, tag="q", name="qc")
                return st["q"]

            def ep_a2(p, mi, ni, nhi):
                for s in range(0, nhi - ni, EPW):
                    e = min(nhi - ni, s + EPW)
                    if (mi + ni // NT + s // EPW) % 2 == 0:
                        nc.scalar.activation(a2_t()[:, mi, ni + s:ni + e],
                                             p[:, s:e], AF.Relu, bias=b2a[mi][:])
                    else:
                        nc.vector.tensor_scalar(a2_t()[:, mi, ni + s:ni + e],
                                                p[:, s:e], b2a[mi][:], 0.0,
                                                ALU.add, ALU.max)

            def ep_q(p, mi, ni, nhi):
                for s in range(0, nhi - ni, EPW):
                    e = min(nhi - ni, s + EPW)
                    if (ni // NT + s // EPW) % 2 == 0:
                        nc.vector.tensor_scalar_add(q_t()[:, ni + s:ni + e],
                                                    p[:, s:e], b2b[0][:])
                    else:
                        nc.scalar.activation(q_t()[:, ni + s:ni + e],
                                             p[:, s:e], AF.Identity,
                                             bias=b2b[0][:])

            h_sb = h_tiles[c]
            quanta += gemm_quanta(w2aT, lambda k: h_sb[:, k, :], H1, ep_a2)
            quanta += gemm_quanta(w2bT, lambda k: st["a2"][:, k, :], OUT_DIM, ep_q)

            def q_dma():
                nc.sync.dma_start(q_d[:, t0 * B:(t0 + TC) * B], st["q"][:])
            quanta.append(q_dma)
            return quanta

        # ------------------------------------------------------------------
        # GRU scan step
        # ------------------------------------------------------------------
        h_prev = {"rhs": None, "ap2": None}  # rhs(k)->[128,B]; ap2 -> [128,(2,B)]

        def scan_step(c, ti):
            t = c * TC + ti
            cb = ti * B
            gi = p1_state[c]["gi"]
            h_sb = h_tiles[c]
            gi_rz = gi[:, 0:4, cb:cb + B]
            gi_n = gi[:, 4:6, cb:cb + B]

            if t == 0:
                r_t = gpool.tile([128, 2 * B], F32, tag="r_t")
                nc.scalar.activation(r_t[:], gi[:, 0:2, cb:cb + B], AF.Sigmoid)
                z_t = gpool.tile([128, 2 * B], F32, tag="z_t")
                nc.scalar.activation(z_t[:], gi[:, 2:4, cb:cb + B], AF.Sigmoid)
                w_n_ap = bhh_bc[:]
            else:
                p1 = psrz.tile([128, 4 * B], F32, tag="psrz")
                p2 = psn.tile([128, 2 * B], F32, tag="psn")
                nc.tensor.matmul(p1[:], ident[:], gi_rz, start=True, stop=False)
                for j in range(4):
                    for k in range(2):
                        nc.tensor.matmul(
                            p1[:, j * B:(j + 1) * B],
                            whhT[k][:, j * 128:(j + 1) * 128],
                            h_prev["rhs"](k),
                            start=False,
                            stop=(j == 3 and k == 1),
                        )
                nc.tensor.matmul(p2[:], bhh2[:], ind2[:], start=True, stop=False)
                for j in range(4, 6):
                    for k in range(2):
                        nc.tensor.matmul(
                            p2[:, (j - 4) * B:(j - 3) * B],
                            whhT[k][:, j * 128:(j + 1) * 128],
                            h_prev["rhs"](k),
                            start=False,
                            stop=(j == 5 and k == 1),
                        )
                # r first: the n-gate chain only needs r; z computes in the
                # shadow of the u/v muls
                r_t = gpool.tile([128, 2 * B], F32, tag="r_t")
                nc.scalar.activation(r_t[:], p1[:, 0:2 * B], AF.Sigmoid)
                z_t = gpool.tile([128, 2 * B], F32, tag="z_t")
                nc.scalar.activation(z_t[:], p1[:, 2 * B:4 * B], AF.Sigmoid)
                # move gh_n out of PSUM while the sigmoid runs; the chained
                # mul then reads SBUF (58cyc access) instead of PSUM (120cyc)
                wn = gpool.tile([128, 2 * B], F32, tag="wn")
                nc.vector.tensor_copy(wn[:], p2[:])
                w_n_ap = wn[:]

            r_ap = r_t[:]
            z_ap = z_t[:]
            u = gpool.tile([128, 2 * B], F32, tag="u")
            nc.vector.tensor_mul(u[:], r_ap, w_n_ap)
            v = gpool.tile([128, 2 * B], F32, tag="v")
            nc.vector.tensor_add(v[:], u[:], gi_n)
            n_t = gpool.tile([128, 2 * B], F32, tag="n_t")
            nc.scalar.activation(n_t[:], v[:], AF.Tanh)

            h_out = h_sb[:, :, cb:cb + B]  # [128, (2, B)] bf16
            if t == 0:
                nc.vector._custom_dve(blend_op, out=h_out, in0=n_t[:], in1=z_ap)
            else:
                zh = gpool.tile([128, 2 * B], F32, tag="zh")
                nc.vector.tensor_mul(zh[:], z_ap, h_prev["ap2"])
                g = gpool.tile([128, 2 * B], F32, tag="g")
                nc.vector._custom_dve(blend_op, out=g[:], in0=n_t[:], in1=z_ap)
                nc.vector.tensor_add(h_out, g[:], zh[:])

            def mk_rhs(hs=h_sb, col=cb):
                return lambda k: hs[:, k, col:col + B]
            h_prev["rhs"] = mk_rhs()
            h_prev["ap2"] = h_sb[:, :, cb:cb + B]

        # ------------------------------------------------------------------
        # pipeline: prologue phase1(0), then scan(c) with phase1(c+1) and
        # phase3(c-1) pumped into the gaps
        # ------------------------------------------------------------------
        def pump(queue, k):
            for _ in range(k):
                if queue:
                    queue.pop(0)()

        def interleave(a, b):
            out = []
            for i in range(max(len(a), len(b))):
                if i < len(a):
                    out.append(a[i])
                if i < len(b):
                    out.append(b[i])
            return out

        prologue = make_xdma(0) + (make_xdma(1) if NCH > 1 else []) + make_phase1(0)
        pump(prologue, len(prologue))

        for c in range(NCH):
            h_tiles[c] = hpool.tile([128, 2, N], BF16, tag="hchunk", name="hchunk")
            queue = list(make_xdma(c + 2)) if c + 2 < NCH else []
            rest = make_phase1(c + 1) if c + 1 < NCH else []
            if c >= 1:
                rest = interleave(rest, make_phase3(c - 1))
            queue += rest
            per_step = (len(queue) + TC - 1) // TC
            for ti in range(TC):
                scan_step(c, ti)
                pump(queue, per_step)
            pump(queue, len(queue))
            if c >= 2:
                del p1_state[c - 2]

        tail = make_phase3(NCH - 1)
        pump(tail, len(tail))

    nc.compile()
    return nc


def kernel(**inputs) -> np.ndarray:
    from concourse.bass_utils import run_bass_kernel_spmd

    x = np.asarray(inputs["x"], dtype=np.float32)
    T, B, _ = x.shape
    b_loc = B // N_CORES

    nc = build_drqn(T=T, B=b_loc, TC=min(32, T))

    f32 = np.float32
    ws = {
        "W1aT": np.ascontiguousarray(np.asarray(inputs["W_fc1a"], f32).T),
        "W1bT": np.ascontiguousarray(np.asarray(inputs["W_fc1b"], f32).T),
        "WihT": np.ascontiguousarray(np.asarray(inputs["W_ih"], f32).T),
        "WhhT": np.ascontiguousarray(np.asarray(inputs["W_hh"], f32).T),
        "W2aT": np.ascontiguousarray(np.asarray(inputs["W_fc2a"], f32).T),
        "W2bT": np.ascontiguousarray(np.asarray(inputs["W_fc2b"], f32).T),
        "b_fc1a": np.asarray(inputs["b_fc1a"], f32),
        "b_fc1b": np.asarray(inputs["b_fc1b"], f32),
        "b_ih": np.asarray(inputs["b_ih"], f32),
        "b_hh": np.asarray(inputs["b_hh"], f32),
        "b_fc2a": np.asarray(inputs["b_fc2a"], f32),
        "b_fc2b": np.asarray(inputs["b_fc2b"], f32),
    }

    in_maps = []
    for c in range(N_CORES):
        shard = x[:, c * b_loc:(c + 1) * b_loc, :]          # [T, b, K]
        xT = np.ascontiguousarray(shard.transpose(2, 0, 1).reshape(IN_DIM, T * b_loc))
        in_maps.append({"xT": xT, **ws})

    res = run_bass_kernel_spmd(nc, in_maps, list(range(N_CORES)))
    outs = []
    for r in res.results:
        qT = r["qT"].reshape(OUT_DIM, T, b_loc)             # [18, T, b]
        outs.append(qT.transpose(1, 2, 0))                  # [T, b, 18]
    out = np.concatenate(outs, axis=1)
    return np.ascontiguousarray(out).astype(np.float32)


def np_ref(ins):
    x = ins["x"]
    relu = lambda v: np.maximum(v, 0.0)
    sig = lambda v: 1.0 / (1.0 + np.exp(-v))
    h = np.zeros((x.shape[1], H), np.float32)
    qs = []
    for t in range(x.shape[0]):
        a = relu(relu(x[t] @ ins["W_fc1a"].T + ins["b_fc1a"]) @ ins["W_fc1b"].T + ins["b_fc1b"])
        gi = a @ ins["W_ih"].T + ins["b_ih"]
        gh = h @ ins["W_hh"].T + ins["b_hh"]
        i_r, i_z, i_n = np.split(gi, 3, -1)
        h_r, h_z, h_n = np.split(gh, 3, -1)
        r = sig(i_r + h_r)
        z = sig(i_z + h_z)
        n = np.tanh(i_n + r * h_n)
        h = (1 - z) * n + z * h
        qs.append(relu(h @ ins["W_fc2a"].T + ins["b_fc2a"]) @ ins["W_fc2b"].T + ins["b_fc2b"])
    return np.stack(qs)


if __name__ == "__main__":
    rng = np.random.default_rng(0)
    s = 0.05
    T, B = 8, 256
    ins = {
        "x": rng.standard_normal((T, B, IN_DIM), dtype=np.float32),
        "W_fc1a": rng.standard_normal((H1, IN_DIM), dtype=np.float32) * s,
        "b_fc1a": rng.standard_normal((H1,), dtype=np.float32) * s,
        "W_fc1b": rng.standard_normal((H, H1), dtype=np.float32) * s,
        "b_fc1b": rng.standard_normal((H,), dtype=np.float32) * s,
        "W_ih": rng.standard_normal((G, H), dtype=np.float32) * s,
        "b_ih": rng.standard_normal((G,), dtype=np.float32) * s,
        "W_hh": rng.standard_normal((G, H), dtype=np.float32) * s,
        "b_hh": rng.standard_normal((G,), dtype=np.float32) * s,
        "W_fc2a": rng.standard_normal((H1, H), dtype=np.float32) * s,
        "b_fc2a": rng.standard_normal((H1,), dtype=np.float32) * s,
        "W_fc2b": rng.standard_normal((OUT_DIM, H1), dtype=np.float32) * s,
        "b_fc2b": rng.standard_normal((OUT_DIM,), dtype=np.float32) * s,
    }
    print("building tiny kernel...")
    expected = np_ref(ins)
    actual = kernel(**ins)
    err = np.abs(actual - expected)
    rel = np.linalg.norm(actual - expected) / np.linalg.norm(expected)
    print("max abs err:", err.max(), "rel l2:", rel)
